# revision 19
# baseline (speedup 1.0000x reference)
"""De-stationary attention on 8 Trainium2 NeuronCores — ACT-bound pipeline.

Problem: y = softmax((x Wq^T + bq)(x Wk^T + bk)^T * scale / (tau*x_std)) (x Wv^T + bv) Wo^T + bo
Shapes: x [4, 2048, 1024], 16 heads of 64 dims, tau=1, delta=0.

Sharding: core c handles batch b = c//2, head group g = c%2 (8 heads).
s = SCALE/x_std[b] is folded into Wq/bq on the host. Host sums the two
head-group partial y's per batch and adds bo + bv @ Wo^T.

Core schedule (the scalar engine's exp stream is the bottleneck at
~1113 ns per [128,1024] tile x 256 = 285 us; everything else hides
under it):
  - x, Wq/Wk/Wv/Wo are bf16 (rel err ~7e-3, budget 2e-2); qT/kT/S
    stay fp32(r).
  - S^T tiles [128 keys, 512 q] computed per head-pair with K=64
    row-tiled matmuls (tile_position (0,0)/(64,0) auto-derived from
    base partitions) -> both heads' S matmuls run concurrently.
  - exp: one ACTIVATE per tk over [128, 1024] psum (both heads),
    double-buffered; output P in bf16.
  - PV: per head, stationary [v | ones] [128, 65] bf16 -> psum
    [65, 512] accumulated over 16 tk; row 64 = softmax denominator l.
  - Normalize: stage O+l out of psum fast, then l -> DRAM -> [64,16]
    -> DVE reciprocal -> DRAM -> broadcast-read -> multiply (bf16).
  - Blocks j-outer (head pair), tq-block rotated b = (bb+j)%4 so the
    output projection for block b unlocks early; projections for
    pairs 1-3, V projection, and output-projection chunks are fed as
    PE "filler" inside the exp-paced loop. Each block's tail PVs and
    normalization are deferred into the next block so the next S pair
    (which feeds the exp stream) issues first.
Emission order = dependency order: Tile resolves deps at creation
time, so a producer emitted after its consumer becomes a WAR hazard.
PSUM: ps_s 2x[128,1024] (4 banks) + ps_o 2x[65,512] (2) + filler
2x[128,512] (2) = 8 banks.
"""

import os
import sys

for _p in ("/opt/trn_rl_repo", "/root/.axon_site/_ro/trn_rl_repo"):
    if os.path.isdir(_p) and _p not in sys.path:
        sys.path.insert(0, _p)

import numpy as np
import ml_dtypes

import concourse.bass as bass
import concourse.mybir as mybir
import concourse.tile as tile
from concourse import bacc
from concourse.bass_utils import run_bass_kernel_spmd

F32 = mybir.dt.float32
F32R = mybir.dt.float32r
BF16 = mybir.dt.bfloat16
AF = mybir.ActivationFunctionType

B, T, D, H = 4, 2048, 1024, 16
HD = D // H          # 64
SCALE = HD ** -0.5
HG = H // 2          # 8 heads per core
EG = HG * HD         # 512 projection dims per core
N_CORES = 8

NTK = T // 128       # 16 key tiles
NB = T // 512        # 4 query blocks of 512
NJ = HG // 2         # 4 head pairs per core
VS = HD + 2          # 66: v cols + ones col + pad (keeps bf16 4B align)


def _build():
    nc = bacc.Bacc("TRN2", target_bir_lowering=False, debug=False)

    xt_d = nc.dram_tensor("xt", [D, T], BF16, kind="ExternalInput")
    # weights are pre-shuffled on the host into the SBUF layout
    # [128, k_chunk * cols] so each load is one contiguous-per-partition
    # DMA (the naive strided layout costs ~25us in 1KB descriptors).
    wq_d = nc.dram_tensor("wq", [128, 8 * EG], BF16, kind="ExternalInput")
    wk_d = nc.dram_tensor("wk", [128, 8 * EG], BF16, kind="ExternalInput")
    wv_d = nc.dram_tensor("wv", [128, 8 * EG], BF16, kind="ExternalInput")
    wo_d = nc.dram_tensor("wo", [128, NJ * D], BF16, kind="ExternalInput")
    bq_d = nc.dram_tensor("bq", [128, NJ], F32, kind="ExternalInput")
    y_d = nc.dram_tensor("y", [T, D], BF16, kind="ExternalOutput")
    # scratch for the softmax-denominator reshape bounce: recip on [1,512]
    # costs 3.3us on one DVE lane; bounced to [64,16] it costs ~0.2us.
    ls_d = nc.dram_tensor("l_scratch", [NJ, NB, 2, 512], F32)
    rs_d = nc.dram_tensor("r_scratch", [NJ, NB, 2, 512], F32)

    with tile.TileContext(nc) as tc:
        from contextlib import ExitStack
        with ExitStack() as octx:
            main = octx.enter_context(tc.tile_pool(name="main", bufs=1))

            qT = [main.tile([128, T], F32R, name=f"qT{j}", tag=f"qT{j}")
                  for j in range(NJ)]
            kTp = [main.tile([128, T], F32R, name=f"kT{j}", tag=f"kT{j}")
                   for j in range(NJ)]
            v_sb = [main.tile([128, HG * VS], BF16, name=f"v{t}", tag=f"v{t}")
                    for t in range(NTK)]
            o_sb = [main.tile([128, T], BF16, name=f"o{j}", tag=f"o{j}")
                    for j in range(NJ)]
            x_t = [main.tile([128, T], BF16, name=f"x{k}", tag=f"x{k}")
                   for k in range(8)]
            # weight k-chunk k lives at cols [k*EG, (k+1)*EG)
            wq_t = main.tile([128, 8 * EG], BF16, name="wq", tag="wq")
            wk_t = main.tile([128, 8 * EG], BF16, name="wk", tag="wk")
            wv_t = main.tile([128, 8 * EG], BF16, name="wv", tag="wv")
            # wo pair j at cols [j*D, (j+1)*D)
            wo_t = main.tile([128, NJ * D], BF16, name="wo", tag="wo")
            bq_t = main.tile([128, NJ], F32, name="bq", tag="bq")

            # ---- input DMAs ----
            # x: quarter 0 per k-chunk first (unblocks pair-0 projections
            # early), then the rest; on the sync queue.
            for k in range(8):
                nc.sync.dma_start(
                    x_t[k][:, 0:512],
                    xt_d.ap()[k * 128:(k + 1) * 128, 0:512])
            for k in range(8):
                nc.sync.dma_start(
                    x_t[k][:, 512:T],
                    xt_d.ap()[k * 128:(k + 1) * 128, 512:T])
            # weights: one contiguous DMA each on the scalar queue.
            nc.scalar.dma_start(wk_t[:], wk_d.ap())
            nc.scalar.dma_start(wq_t[:], wq_d.ap())
            nc.scalar.dma_start(bq_t[:], bq_d.ap())
            nc.scalar.dma_start(wv_t[:], wv_d.ap())
            nc.scalar.dma_start(wo_t[:], wo_d.ap())

            # ones column of each v tile (col 64 of each 66-stride group;
            # col 65 is pad, set to 1.0 too so the tile is fully init'd)
            for t in range(NTK):
                vv = v_sb[t][:].rearrange("p (h c) -> p h c", c=VS)
                nc.vector.memset(vv[:, :, HD:HD + 2], 1.0)

            # ---- warm-up: trip the PE HAM + preload exp table ----
            with tc.tile_pool(name="wu", bufs=1) as wup, \
                 tc.tile_pool(name="wu_ps", bufs=1, space="PSUM") as wups:
                wu_t = wup.tile([128, 512], BF16, name="wu")
                nc.vector.memset(wu_t[:], 0.0)
                wu_e = wup.tile([128, 8], F32, name="wue")
                nc.scalar.activation(wu_e[:], wu_t[:, 0:8], AF.Exp)
                wu_ps = wups.tile([128, 512], F32, name="wups")
                for i in range(40):
                    nc.tensor.matmul(wu_ps[:], wu_t[:, 0:128], wu_t[:],
                                     start=True, stop=True,
                                     skip_group_check=True)

            with tc.tile_pool(name="sp", bufs=1, space="PSUM") as spool, \
                 tc.tile_pool(name="op", bufs=1, space="PSUM") as opool, \
                 tc.tile_pool(name="fp", bufs=1, space="PSUM") as fpool, \
                 tc.tile_pool(name="pts", bufs=1) as ptpool, \
                 tc.tile_pool(name="aux", bufs=1) as aux:

                def proj_chunk(kind, j, q):
                    qsl = slice(q * 512, (q + 1) * 512)
                    ps = fpool.tile([128, 512], F32, name="fps", tag="fps",
                                    bufs=2)
                    wt = wk_t if kind == 'k' else wq_t
                    for k in range(8):
                        nc.tensor.matmul(
                            ps[:],
                            wt[:, k * EG + j * 128:k * EG + (j + 1) * 128],
                            x_t[k][:, qsl],
                            start=(k == 0), stop=(k == 7))
                    if kind == 'k':
                        nc.vector.tensor_copy(kTp[j][:, qsl], ps[:])
                    else:
                        nc.vector.tensor_scalar_add(qT[j][:, qsl], ps[:],
                                                    bq_t[:, j:j + 1])

                def v_chunk(ti):
                    ssl = slice(ti * 128, (ti + 1) * 128)
                    ps = fpool.tile([128, 512], F32, name="fps", tag="fps",
                                    bufs=2)
                    for k in range(8):
                        nc.tensor.matmul(
                            ps[:], x_t[k][:, ssl],
                            wv_t[:, k * EG:(k + 1) * EG],
                            start=(k == 0), stop=(k == 7))
                    vv = v_sb[ti][:].rearrange("p (h c) -> p h c", c=VS)
                    nc.vector.tensor_copy(
                        vv[:, :, 0:HD],
                        ps[:].rearrange("p (h c) -> p h c", c=HD))

                def c_chunk(tt, half):
                    tsl = slice(tt * 128, (tt + 1) * 128)
                    nsl = slice(half * 512, (half + 1) * 512)
                    ps = fpool.tile([128, 512], F32, name="fps", tag="fps",
                                    bufs=2)
                    for j in range(NJ):
                        nc.tensor.matmul(ps[:], o_sb[j][:, tsl],
                                         wo_t[:, j * D + half * 512:
                                              j * D + (half + 1) * 512],
                                         start=(j == 0), stop=(j == NJ - 1))
                    yt = aux.tile([128, 512], BF16, name="yt", tag="yt",
                                  bufs=3)
                    nc.vector.tensor_copy(yt[:], ps[:])
                    nc.sync.dma_start(y_d.ap()[tsl, nsl], yt[:])

                # ---- startup: just k/q for pair 0 quarter 0; the rest
                # rides the filler stream (k(0,q) feeds the S pace of the
                # first block, v feeds its PVs, q(0,q) is needed from
                # block 2 on).
                proj_chunk('k', 0, 0)
                proj_chunk('q', 0, 0)

                # ---- filler queue ----
                filler = [
                    ('k', 0, 1), ('v', 0, 0), ('v', 1, 0),
                    ('k', 0, 2), ('v', 2, 0), ('v', 3, 0),
                    ('k', 0, 3), ('v', 4, 0), ('v', 5, 0),
                    ('q', 0, 1), ('v', 6, 0), ('v', 7, 0),
                    ('q', 0, 2), ('q', 0, 3),
                ]
                filler.extend([('v', t, 0) for t in range(8, NTK)])
                for jf in (1, 2, 3):
                    filler.extend([('k', jf, q) for q in range(4)])
                    filler.extend([('q', jf, q) for q in range(4)])
                fidx = [0]

                def pull_filler(n=1):
                    for _ in range(n):
                        if fidx[0] >= len(filler):
                            return
                        kind, a, bb_ = filler[fidx[0]]
                        fidx[0] += 1
                        if kind == 'v':
                            v_chunk(a)
                        elif kind == 'c':
                            c_chunk(a, bb_)
                        else:
                            proj_chunk(kind, a, bb_)

                def emit_pv(j, tk, ps_o, pts):
                    for h in range(2):
                        g = 2 * j + h
                        vcol = slice(g * VS, g * VS + HD + 1)
                        nc.tensor.matmul(
                            ps_o[h][:],
                            v_sb[tk][:, vcol],
                            pts[tk][:, h * 512:(h + 1) * 512],
                            start=(tk == 0), stop=(tk == NTK - 1))

                done_b = [0] * NB

                def make_tail(j, b, ps_o, pts):
                    bsl = slice(b * 512, (b + 1) * 512)

                    def tail():
                        emit_pv(j, NTK - 2, ps_o, pts)
                        emit_pv(j, NTK - 1, ps_o, pts)
                        # stage O+l out of psum promptly, then normalize
                        # out-of-band via the DRAM reshape bounce.
                        stage = [aux.tile([65, 512], F32, name=f"st{h}",
                                          tag=f"st{h}", bufs=2)
                                 for h in range(2)]
                        for h in range(2):
                            nc.vector.tensor_copy(stage[h][:], ps_o[h][:])
                            nc.sync.dma_start(ls_d.ap()[j, b, h, :],
                                              stage[h][64:65, :])
                        lr = aux.tile([64, 16], F32, name="lr", tag="lr",
                                      bufs=2)
                        nc.sync.dma_start(
                            lr[:],
                            ls_d.ap()[j, b].rearrange("h (a c) -> a h c",
                                                      c=8))
                        rr = aux.tile([64, 16], F32, name="rr", tag="rr",
                                      bufs=2)
                        nc.vector.reciprocal(rr[:], lr[:])
                        nc.sync.dma_start(
                            rs_d.ap()[j, b].rearrange("h (a c) -> a h c",
                                                      c=8), rr[:])
                        rbc = aux.tile([64, 1024], F32, name="rbc",
                                       tag="rbc", bufs=2)
                        nc.sync.dma_start(
                            rbc[:].rearrange("p (h c) -> p h c", c=512),
                            rs_d.ap()[j, b][None, :, :]
                            .broadcast_to((64, 2, 512)))
                        for h in range(2):
                            nc.vector.tensor_mul(
                                o_sb[j][h * 64:(h + 1) * 64, bsl],
                                stage[h][0:64, :],
                                rbc[:, h * 512:(h + 1) * 512])
                        done_b[b] += 1
                        if done_b[b] == NJ:
                            filler.extend([('c', tt, half)
                                           for tt in range(b * 4, b * 4 + 4)
                                           for half in range(2)])
                    return tail

                # ---- attention blocks ----
                pending_tail = None
                for j in range(NJ):
                    for bb in range(NB):
                        b = (bb + j) % NB
                        bsl = slice(b * 512, (b + 1) * 512)
                        ps_o = None
                        pts = [None] * NTK

                        for p in range(NTK // 2):
                            for u in range(2):
                                tk = 2 * p + u
                                ksl = slice(tk * 128, (tk + 1) * 128)
                                ps_s = spool.tile([128, 1024], F32,
                                                  name="ps_s", tag="ps_s",
                                                  bufs=2)
                                for h in range(2):
                                    hp = slice(h * 64, (h + 1) * 64)
                                    nc.tensor.matmul(
                                        ps_s[:, h * 512:(h + 1) * 512],
                                        kTp[j][hp, ksl], qT[j][hp, bsl],
                                        start=True, stop=True)
                                pts[tk] = ptpool.tile([128, 1024], BF16,
                                                      name="pt", tag="pt",
                                                      bufs=10)
                                nc.scalar.activation(pts[tk][:], ps_s[:],
                                                     AF.Exp)
                            if p == 0 and pending_tail is not None:
                                # previous block's tail PVs + normalize run
                                # after this block's first S pair is queued.
                                pending_tail()
                                pending_tail = None
                            # first blocks must emit chunks ahead of
                            # their consumers: deps are created in
                            # emission order. Late blocks drain C chunks.
                            nblk = j * NB + bb
                            pull_filler(3 if nblk == 0 else
                                        (2 if nblk in (1, 13, 14, 15)
                                         else 1))
                            if p == 1:
                                ps_o = [opool.tile([65, 512], F32,
                                                   name=f"po{h}",
                                                   tag=f"po{h}", bufs=1)
                                        for h in range(2)]
                            if p > 0:
                                emit_pv(j, 2 * p - 2, ps_o, pts)
                                emit_pv(j, 2 * p - 1, ps_o, pts)
                        pending_tail = make_tail(j, b, ps_o, pts)

                pending_tail()
                pending_tail = None

                # ---- drain remaining filler (last C chunks) ----
                while fidx[0] < len(filler):
                    pull_filler()

    nc.compile()
    return nc


_NC = None
_last_in_maps = None


def kernel(x, x_mean, x_std, Wq, bq, Wk, bk, Wv, bv, Wo, bo):
    global _NC
    if _NC is None:
        _NC = _build()

    bf = ml_dtypes.bfloat16
    x = np.asarray(x, dtype=np.float32)
    x_std = np.asarray(x_std, dtype=np.float32)
    Wq = np.asarray(Wq, dtype=np.float32)
    Wk = np.asarray(Wk, dtype=np.float32)
    Wv = np.asarray(Wv, dtype=np.float32)
    Wo = np.asarray(Wo, dtype=np.float32)
    bq = np.asarray(bq, dtype=np.float32)
    bv = np.asarray(bv, dtype=np.float32)
    bo = np.asarray(bo, dtype=np.float32)

    in_maps = []
    for c in range(N_CORES):
        b, g = c // 2, c % 2
        s = np.float32(SCALE / float(x_std[b, 0, 0]))
        rows = slice(g * EG, (g + 1) * EG)
        def shuf(wt, nchunk, cols):
            # [nchunk*128, cols] -> [128, nchunk*cols] (k-chunk-major cols)
            return np.ascontiguousarray(
                wt.reshape(nchunk, 128, cols).transpose(1, 0, 2)
                .reshape(128, nchunk * cols))
        in_maps.append({
            "xt": np.ascontiguousarray(x[b].T).astype(bf),
            "wq": shuf((Wq[rows, :] * s).T, 8, EG).astype(bf),
            "wk": shuf(Wk[rows, :].T, 8, EG).astype(bf),
            "wv": shuf(Wv[rows, :].T, 8, EG).astype(bf),
            "wo": shuf(Wo[:, rows].T, NJ, D).astype(bf),
            "bq": np.ascontiguousarray((bq[rows] * s).reshape(NJ, 128).T),
        })

    global _last_in_maps
    _last_in_maps = in_maps
    res = run_bass_kernel_spmd(_NC, in_maps, list(range(N_CORES)))

    bias_term = (bo + bv @ Wo.T).astype(np.float32)   # [D]
    y = np.empty((B, T, D), dtype=np.float32)
    for b in range(B):
        y[b] = (res.results[2 * b]["y"].astype(np.float32)
                + res.results[2 * b + 1]["y"].astype(np.float32)
                + bias_term[None, :])
    return y


# revision 20
# speedup vs baseline: 1.0511x; 1.0511x over previous
"""De-stationary attention on 8 Trainium2 NeuronCores — ACT-bound pipeline.

Problem: y = softmax((x Wq^T + bq)(x Wk^T + bk)^T * scale / (tau*x_std)) (x Wv^T + bv) Wo^T + bo
Shapes: x [4, 2048, 1024], 16 heads of 64 dims, tau=1, delta=0.

Sharding: core c handles batch b = c//2, head group g = c%2 (8 heads).
s = SCALE/x_std[b] is folded into Wq/bq on the host. Host sums the two
head-group partial y's per batch and adds bo + bv @ Wo^T.

Core schedule (the scalar engine's exp stream is the bottleneck at
~1113 ns per [128,1024] tile x 256 = 285 us; everything else hides
under it):
  - x, Wq/Wk/Wv/Wo are bf16 (rel err ~7e-3, budget 2e-2); qT/kT/S
    stay fp32(r).
  - S^T tiles [128 keys, 512 q] computed per head-pair with K=64
    row-tiled matmuls (tile_position (0,0)/(64,0) auto-derived from
    base partitions) -> both heads' S matmuls run concurrently.
  - exp: one ACTIVATE per tk over [128, 1024] psum (both heads),
    double-buffered; output P in bf16.
  - PV: per head, stationary [v | ones] [128, 65] bf16 -> psum
    [65, 512] accumulated over 16 tk; row 64 = softmax denominator l.
  - Normalize: stage O+l out of psum fast, then l -> DRAM -> [64,16]
    -> DVE reciprocal -> DRAM -> broadcast-read -> multiply (bf16).
  - Blocks j-outer (head pair), tq-block rotated b = (bb+j)%4 so the
    output projection for block b unlocks early; projections for
    pairs 1-3, V projection, and output-projection chunks are fed as
    PE "filler" inside the exp-paced loop. Each block's tail PVs and
    normalization are deferred into the next block so the next S pair
    (which feeds the exp stream) issues first.
Emission order = dependency order: Tile resolves deps at creation
time, so a producer emitted after its consumer becomes a WAR hazard.
PSUM: ps_s 2x[128,1024] (4 banks) + ps_o 2x[65,512] (2) + filler
2x[128,512] (2) = 8 banks.
"""

import os
import sys

for _p in ("/opt/trn_rl_repo", "/root/.axon_site/_ro/trn_rl_repo"):
    if os.path.isdir(_p) and _p not in sys.path:
        sys.path.insert(0, _p)

import numpy as np
import ml_dtypes

import concourse.bass as bass
import concourse.mybir as mybir
import concourse.tile as tile
from concourse import bacc
from concourse.bass_utils import run_bass_kernel_spmd

F32 = mybir.dt.float32
F32R = mybir.dt.float32r
BF16 = mybir.dt.bfloat16
AF = mybir.ActivationFunctionType

B, T, D, H = 4, 2048, 1024, 16
HD = D // H          # 64
SCALE = HD ** -0.5
HG = H // 2          # 8 heads per core
EG = HG * HD         # 512 projection dims per core
N_CORES = 8

NTK = T // 128       # 16 key tiles
NB = T // 512        # 4 query blocks of 512
NJ = HG // 2         # 4 head pairs per core
VS = HD + 2          # 66: v cols + ones col + pad (keeps bf16 4B align)


def _build():
    nc = bacc.Bacc("TRN2", target_bir_lowering=False, debug=False)

    xt_d = nc.dram_tensor("xt", [D, T], BF16, kind="ExternalInput")
    # weights are pre-shuffled on the host into the SBUF layout
    # [128, k_chunk * cols] so each load is one contiguous-per-partition
    # DMA (the naive strided layout costs ~25us in 1KB descriptors).
    wq_d = nc.dram_tensor("wq", [128, 8 * EG], BF16, kind="ExternalInput")
    wk_d = nc.dram_tensor("wk", [128, 8 * EG], BF16, kind="ExternalInput")
    wv_d = nc.dram_tensor("wv", [128, 8 * EG], BF16, kind="ExternalInput")
    wo_d = nc.dram_tensor("wo", [128, NJ * D], BF16, kind="ExternalInput")
    bq_d = nc.dram_tensor("bq", [128, NJ], F32, kind="ExternalInput")
    y_d = nc.dram_tensor("y", [T, D], BF16, kind="ExternalOutput")
    # scratch for the softmax-denominator reshape bounce: recip on [1,512]
    # costs 3.3us on one DVE lane; bounced to [64,16] it costs ~0.2us.
    ls_d = nc.dram_tensor("l_scratch", [NJ, NB, 2, 512], F32)
    rs_d = nc.dram_tensor("r_scratch", [NJ, NB, 2, 512], F32)

    with tile.TileContext(nc) as tc:
        from contextlib import ExitStack
        with ExitStack() as octx:
            main = octx.enter_context(tc.tile_pool(name="main", bufs=1))

            qT = [main.tile([128, T], F32R, name=f"qT{j}", tag=f"qT{j}")
                  for j in range(NJ)]
            kTp = [main.tile([128, T], F32R, name=f"kT{j}", tag=f"kT{j}")
                   for j in range(NJ)]
            v_sb = [main.tile([128, HG * VS], BF16, name=f"v{t}", tag=f"v{t}")
                    for t in range(NTK)]
            o_sb = [main.tile([128, T], BF16, name=f"o{j}", tag=f"o{j}")
                    for j in range(NJ)]
            x_t = [main.tile([128, T], BF16, name=f"x{k}", tag=f"x{k}")
                   for k in range(8)]
            # weight k-chunk k lives at cols [k*EG, (k+1)*EG)
            wq_t = main.tile([128, 8 * EG], BF16, name="wq", tag="wq")
            wk_t = main.tile([128, 8 * EG], BF16, name="wk", tag="wk")
            wv_t = main.tile([128, 8 * EG], BF16, name="wv", tag="wv")
            # wo pair j at cols [j*D, (j+1)*D)
            wo_t = main.tile([128, NJ * D], BF16, name="wo", tag="wo")
            bq_t = main.tile([128, NJ], F32, name="bq", tag="bq")

            # ---- input DMAs ----
            # x: quarter 0 per k-chunk first (unblocks pair-0 projections
            # early), then the rest; on the sync queue.
            for k in range(8):
                nc.sync.dma_start(
                    x_t[k][:, 0:512],
                    xt_d.ap()[k * 128:(k + 1) * 128, 0:512])
            for k in range(8):
                nc.sync.dma_start(
                    x_t[k][:, 512:T],
                    xt_d.ap()[k * 128:(k + 1) * 128, 512:T])
            # weights: one contiguous DMA each on the scalar queue.
            nc.scalar.dma_start(wk_t[:], wk_d.ap())
            nc.scalar.dma_start(wq_t[:], wq_d.ap())
            nc.scalar.dma_start(bq_t[:], bq_d.ap())
            nc.scalar.dma_start(wv_t[:], wv_d.ap())
            nc.scalar.dma_start(wo_t[:], wo_d.ap())

            # ones column of each v tile (col 64 of each 66-stride group;
            # col 65 is pad, set to 1.0 too so the tile is fully init'd)
            for t in range(NTK):
                vv = v_sb[t][:].rearrange("p (h c) -> p h c", c=VS)
                nc.vector.memset(vv[:, :, HD:HD + 2], 1.0)

            # ---- warm-up: trip the PE HAM + preload exp table ----
            with tc.tile_pool(name="wu", bufs=1) as wup, \
                 tc.tile_pool(name="wu_ps", bufs=1, space="PSUM") as wups:
                wu_t = wup.tile([128, 512], BF16, name="wu")
                nc.vector.memset(wu_t[:], 0.0)
                wu_e = wup.tile([128, 8], F32, name="wue")
                nc.scalar.activation(wu_e[:], wu_t[:, 0:8], AF.Exp)
                wu_ps = wups.tile([128, 512], F32, name="wups")
                for i in range(40):
                    nc.tensor.matmul(wu_ps[:], wu_t[:, 0:128], wu_t[:],
                                     start=True, stop=True,
                                     skip_group_check=True)

            with tc.tile_pool(name="sp", bufs=1, space="PSUM") as spool, \
                 tc.tile_pool(name="op", bufs=1, space="PSUM") as opool, \
                 tc.tile_pool(name="fp", bufs=1, space="PSUM") as fpool, \
                 tc.tile_pool(name="pts", bufs=1) as ptpool, \
                 tc.tile_pool(name="aux", bufs=1) as aux:

                def proj_chunk(kind, j, q):
                    qsl = slice(q * 512, (q + 1) * 512)
                    ps = fpool.tile([128, 512], F32, name="fps", tag="fps",
                                    bufs=2)
                    wt = wk_t if kind == 'k' else wq_t
                    for k in range(8):
                        nc.tensor.matmul(
                            ps[:],
                            wt[:, k * EG + j * 128:k * EG + (j + 1) * 128],
                            x_t[k][:, qsl],
                            start=(k == 0), stop=(k == 7))
                    if kind == 'k':
                        nc.vector.tensor_copy(kTp[j][:, qsl], ps[:])
                    else:
                        nc.vector.tensor_scalar_add(qT[j][:, qsl], ps[:],
                                                    bq_t[:, j:j + 1])

                def v_chunk(ti):
                    ssl = slice(ti * 128, (ti + 1) * 128)
                    ps = fpool.tile([128, 512], F32, name="fps", tag="fps",
                                    bufs=2)
                    for k in range(8):
                        nc.tensor.matmul(
                            ps[:], x_t[k][:, ssl],
                            wv_t[:, k * EG:(k + 1) * EG],
                            start=(k == 0), stop=(k == 7))
                    vv = v_sb[ti][:].rearrange("p (h c) -> p h c", c=VS)
                    nc.vector.tensor_copy(
                        vv[:, :, 0:HD],
                        ps[:].rearrange("p (h c) -> p h c", c=HD))

                def c_chunk(tt, half):
                    tsl = slice(tt * 128, (tt + 1) * 128)
                    nsl = slice(half * 512, (half + 1) * 512)
                    ps = fpool.tile([128, 512], F32, name="fps", tag="fps",
                                    bufs=2)
                    for j in range(NJ):
                        nc.tensor.matmul(ps[:], o_sb[j][:, tsl],
                                         wo_t[:, j * D + half * 512:
                                              j * D + (half + 1) * 512],
                                         start=(j == 0), stop=(j == NJ - 1))
                    yt = aux.tile([128, 512], BF16, name="yt", tag="yt",
                                  bufs=3)
                    nc.vector.tensor_copy(yt[:], ps[:])
                    nc.sync.dma_start(y_d.ap()[tsl, nsl], yt[:])

                # ---- startup: k/q for pair 0 quarter 0 (unblocks the
                # exp stream) plus the first half of the V projection
                # (the first block's PVs consume v faster than filler
                # slots can produce it).
                proj_chunk('k', 0, 0)
                proj_chunk('q', 0, 0)
                for t in range(8):
                    v_chunk(t)

                # ---- filler queue ----
                filler = [
                    ('k', 0, 1), ('v', 8, 0), ('v', 9, 0),
                    ('k', 0, 2), ('v', 10, 0), ('v', 11, 0),
                    ('k', 0, 3), ('v', 12, 0), ('v', 13, 0),
                    ('v', 14, 0), ('v', 15, 0),
                    ('q', 0, 1), ('q', 0, 2), ('q', 0, 3),
                ]
                for jf in (1, 2, 3):
                    filler.extend([('k', jf, q) for q in range(4)])
                    filler.extend([('q', jf, q) for q in range(4)])
                fidx = [0]

                def pull_filler(n=1):
                    for _ in range(n):
                        if fidx[0] >= len(filler):
                            return
                        kind, a, bb_ = filler[fidx[0]]
                        fidx[0] += 1
                        if kind == 'v':
                            v_chunk(a)
                        elif kind == 'c':
                            c_chunk(a, bb_)
                        else:
                            proj_chunk(kind, a, bb_)

                def emit_pv(j, tk, ps_o, pts):
                    for h in range(2):
                        g = 2 * j + h
                        vcol = slice(g * VS, g * VS + HD + 1)
                        nc.tensor.matmul(
                            ps_o[h][:],
                            v_sb[tk][:, vcol],
                            pts[tk][:, h * 512:(h + 1) * 512],
                            start=(tk == 0), stop=(tk == NTK - 1))

                done_b = [0] * NB

                def make_tail(j, b, ps_o, pts):
                    bsl = slice(b * 512, (b + 1) * 512)

                    def tail():
                        emit_pv(j, NTK - 2, ps_o, pts)
                        emit_pv(j, NTK - 1, ps_o, pts)
                        # stage O+l out of psum promptly, then normalize
                        # out-of-band via the DRAM reshape bounce.
                        stage = [aux.tile([65, 512], F32, name=f"st{h}",
                                          tag=f"st{h}", bufs=2)
                                 for h in range(2)]
                        for h in range(2):
                            nc.vector.tensor_copy(stage[h][:], ps_o[h][:])
                            nc.sync.dma_start(ls_d.ap()[j, b, h, :],
                                              stage[h][64:65, :])
                        lr = aux.tile([64, 16], F32, name="lr", tag="lr",
                                      bufs=2)
                        nc.sync.dma_start(
                            lr[:],
                            ls_d.ap()[j, b].rearrange("h (a c) -> a h c",
                                                      c=8))
                        rr = aux.tile([64, 16], F32, name="rr", tag="rr",
                                      bufs=2)
                        nc.vector.reciprocal(rr[:], lr[:])
                        nc.sync.dma_start(
                            rs_d.ap()[j, b].rearrange("h (a c) -> a h c",
                                                      c=8), rr[:])
                        rbc = aux.tile([64, 1024], F32, name="rbc",
                                       tag="rbc", bufs=2)
                        nc.sync.dma_start(
                            rbc[:].rearrange("p (h c) -> p h c", c=512),
                            rs_d.ap()[j, b][None, :, :]
                            .broadcast_to((64, 2, 512)))
                        for h in range(2):
                            nc.vector.tensor_mul(
                                o_sb[j][h * 64:(h + 1) * 64, bsl],
                                stage[h][0:64, :],
                                rbc[:, h * 512:(h + 1) * 512])
                        done_b[b] += 1
                        if done_b[b] == NJ:
                            filler.extend([('c', tt, half)
                                           for tt in range(b * 4, b * 4 + 4)
                                           for half in range(2)])
                    return tail

                # ---- attention blocks ----
                pending_tail = None
                for j in range(NJ):
                    for bb in range(NB):
                        b = (bb + j) % NB
                        bsl = slice(b * 512, (b + 1) * 512)
                        ps_o = None
                        pts = [None] * NTK

                        for p in range(NTK // 2):
                            for u in range(2):
                                tk = 2 * p + u
                                ksl = slice(tk * 128, (tk + 1) * 128)
                                ps_s = spool.tile([128, 1024], F32,
                                                  name="ps_s", tag="ps_s",
                                                  bufs=2)
                                for h in range(2):
                                    hp = slice(h * 64, (h + 1) * 64)
                                    nc.tensor.matmul(
                                        ps_s[:, h * 512:(h + 1) * 512],
                                        kTp[j][hp, ksl], qT[j][hp, bsl],
                                        start=True, stop=True)
                                pts[tk] = ptpool.tile([128, 1024], BF16,
                                                      name="pt", tag="pt",
                                                      bufs=10)
                                nc.scalar.activation(pts[tk][:], ps_s[:],
                                                     AF.Exp)
                            if p == 0 and pending_tail is not None:
                                # previous block's tail PVs + normalize run
                                # after this block's first S pair is queued.
                                pending_tail()
                                pending_tail = None
                            # first blocks must emit chunks ahead of
                            # their consumers: deps are created in
                            # emission order. Late blocks drain C chunks
                            # gently (half the slack) to keep exp paced.
                            nblk = j * NB + bb
                            if nblk == 0:
                                pull_filler(2)
                            elif nblk >= 13 and p % 2 == 0:
                                pull_filler(2)
                            else:
                                pull_filler(1)
                            if p == 1:
                                ps_o = [opool.tile([65, 512], F32,
                                                   name=f"po{h}",
                                                   tag=f"po{h}", bufs=1)
                                        for h in range(2)]
                            if p > 0:
                                emit_pv(j, 2 * p - 2, ps_o, pts)
                                emit_pv(j, 2 * p - 1, ps_o, pts)
                        pending_tail = make_tail(j, b, ps_o, pts)

                pending_tail()
                pending_tail = None

                # ---- drain remaining filler (last C chunks) ----
                while fidx[0] < len(filler):
                    pull_filler()

    nc.compile()
    return nc


_NC = None
_last_in_maps = None


def kernel(x, x_mean, x_std, Wq, bq, Wk, bk, Wv, bv, Wo, bo):
    global _NC
    if _NC is None:
        _NC = _build()

    bf = ml_dtypes.bfloat16
    x = np.asarray(x, dtype=np.float32)
    x_std = np.asarray(x_std, dtype=np.float32)
    Wq = np.asarray(Wq, dtype=np.float32)
    Wk = np.asarray(Wk, dtype=np.float32)
    Wv = np.asarray(Wv, dtype=np.float32)
    Wo = np.asarray(Wo, dtype=np.float32)
    bq = np.asarray(bq, dtype=np.float32)
    bv = np.asarray(bv, dtype=np.float32)
    bo = np.asarray(bo, dtype=np.float32)

    in_maps = []
    for c in range(N_CORES):
        b, g = c // 2, c % 2
        s = np.float32(SCALE / float(x_std[b, 0, 0]))
        rows = slice(g * EG, (g + 1) * EG)
        def shuf(wt, nchunk, cols):
            # [nchunk*128, cols] -> [128, nchunk*cols] (k-chunk-major cols)
            return np.ascontiguousarray(
                wt.reshape(nchunk, 128, cols).transpose(1, 0, 2)
                .reshape(128, nchunk * cols))
        in_maps.append({
            "xt": np.ascontiguousarray(x[b].T).astype(bf),
            "wq": shuf((Wq[rows, :] * s).T, 8, EG).astype(bf),
            "wk": shuf(Wk[rows, :].T, 8, EG).astype(bf),
            "wv": shuf(Wv[rows, :].T, 8, EG).astype(bf),
            "wo": shuf(Wo[:, rows].T, NJ, D).astype(bf),
            "bq": np.ascontiguousarray((bq[rows] * s).reshape(NJ, 128).T),
        })

    global _last_in_maps
    _last_in_maps = in_maps
    res = run_bass_kernel_spmd(_NC, in_maps, list(range(N_CORES)))

    bias_term = (bo + bv @ Wo.T).astype(np.float32)   # [D]
    y = np.empty((B, T, D), dtype=np.float32)
    for b in range(B):
        y[b] = (res.results[2 * b]["y"].astype(np.float32)
                + res.results[2 * b + 1]["y"].astype(np.float32)
                + bias_term[None, :])
    return y
